# revision 1
# baseline (speedup 1.0000x reference)
"""BasicGNNConv on 8 TRN2 NeuronCores (Bass/Tile).

Math (reference):
    h   = node_feat @ Wn + bn                    # [N, 128]
    e   = edge_feat @ We + be                    # [E, 128]
    m   = h[src] + e
    agg = segment_sum(m, dst) / max(deg, 1)
    out = concat([h, agg]) @ Wc + bc

Linearity rewrite (eliminates all per-edge matmuls):
    ht  = node_feat @ Wn                         # h-tilde, no bias
    Sh  = segment_sum(ht[src], dst)
    Sef = segment_sum(edge_feat, dst)            # raw edge features
    agg = (Sh + Sef @ We) / max(deg, 1) + min(deg, 1) * (bn + be)
    out = ht @ Wc1 + agg @ Wc2 + (bn @ Wc1 + bc)

Sharding: edges are assigned to the core that owns their dst node range
(5000 nodes/core) -> per-core segment sums are complete, no collective needed.

Per-core device pipeline:
  A.  ht for all 40000 nodes (replicated compute, bf16) -> HBM gather table.
  A2. ht.T for the core's own 5000 nodes in f32 (kept in SBUF).
  B.  Edge stream: host pre-sorts each core's edges by 125-node dst-block
      (sub-ordered by src<32768 to satisfy dma_gather's int16 index range).
      Per 128-edge tile: dma_gather ht[src] rows (4 SWDGE queues round-robin),
      build the one-hot S[e, n] = (dst_local[e] == n) with a DVE is_equal
      against an iota constant, and accumulate S.T @ msgs into PSUM (exact
      f32 segment sums). Per block, the combine (We / mean / Wc / biases) runs
      as an epilogue accumulating straight onto the open PSUM group, then the
      output slice is DMA'd. deg / 1/max(deg,1) / min(deg,1) are index-side
      metadata computed on host along with the edge sharding.
"""
import os
import numpy as np
import ml_dtypes

import concourse.bacc as bacc
import concourse.mybir as mybir
import concourse.tile as tile
from concourse.tile_rust import add_dep_helper
from concourse.bass_utils import run_bass_kernel_spmd
from concourse.masks import make_identity

N = 40000
E = 640000
D = 128          # OUT_DIM == EDGE_DIM
ND = 256         # NODE_DIM
C = 8            # cores
NPC = N // C     # 5000 nodes per core
BLK = 125        # nodes per dst block
NB = NPC // BLK  # 40 blocks per core
HALF = 32768     # int16 index range split for the ht table
PAD_COL = 127    # trash column in the 128-wide S window (>= BLK)
GMAX = 8         # tiles per dma_gather call (SWDGE ring: 1024 descs/queue)
NQ = 4           # SWDGE queues

LAST_EXEC_NS = None
LAST_RESULTS = None

bf16 = ml_dtypes.bfloat16


def _wrap_idx16(arr):
    """[L] -> [128, L//16] int16 wrapped layout (pos i at [i%16, i//16]),
    replicated across the 8 GPSIMD core partition groups."""
    w = arr.astype(np.int16).reshape(-1, 16).T
    return np.ascontiguousarray(np.tile(w, (8, 1)))


def _build_graph(T_list):
    nc = bacc.Bacc(
        None, target_bir_lowering=False, debug=False, num_swdge_queues=NQ
    )
    f32, i16 = mybir.dt.float32, mybir.dt.int16
    bft = mybir.dt.bfloat16

    T_tot = sum(tl + th for tl, th in T_list)
    L = T_tot * 128

    nfT_b = nc.declare_dram_parameter("nfT_b", [ND, N], bft, isOutput=False)
    nfT_own = nc.declare_dram_parameter("nfT_own", [ND, NPC], f32, isOutput=False)
    Wn_p = nc.declare_dram_parameter("Wn", [ND, D], f32, isOutput=False)
    We_p = nc.declare_dram_parameter("We", [D, D], f32, isOutput=False)
    Wc1_p = nc.declare_dram_parameter("Wc1", [D, D], f32, isOutput=False)
    Wc2_p = nc.declare_dram_parameter("Wc2", [D, D], f32, isOutput=False)
    bias0_p = nc.declare_dram_parameter("bias0", [1, D], f32, isOutput=False)
    rowbnbe_p = nc.declare_dram_parameter("rowbnbe", [128, D], f32, isOutput=False)
    rcol_p = nc.declare_dram_parameter("rcol", [128, NB], f32, isOutput=False)
    mcol_p = nc.declare_dram_parameter("mcol", [128, NB], f32, isOutput=False)
    gidx_p = nc.declare_dram_parameter("gidx", [128, L // 16], i16, isOutput=False)
    dstl_p = nc.declare_dram_parameter("dstl", [128, T_tot], i16, isOutput=False)
    ef_p = nc.declare_dram_parameter("ef", [128, T_tot, D], bft, isOutput=False)
    out_p = nc.declare_dram_parameter("out", [NPC, D], f32, isOutput=True)

    htab_lo = nc.dram_tensor("htab_lo", [HALF, D], bft)
    htab_hi = nc.dram_tensor("htab_hi", [N - HALF, D], bft)

    with tile.TileContext(nc) as tc:
        with (
            tc.tile_pool(name="const", bufs=1) as cpool,
            tc.tile_pool(name="tabs", bufs=1) as tpool,
        ):
            # ---- constants / weights in SBUF ----
            ident = cpool.tile([128, 128], f32)
            make_identity(nc, ident[:])
            iota_i = cpool.tile([128, 128], mybir.dt.int32)
            nc.gpsimd.iota(iota_i[:], pattern=[[1, 128]], base=0, channel_multiplier=0)
            iota_b = cpool.tile([128, 128], bft)
            nc.vector.tensor_copy(iota_b[:], iota_i[:])
            iota4 = cpool.tile([128, 4, 128], bft)
            for j in range(4):
                nc.vector.tensor_copy(iota4[:, j, :], iota_i[:])
            ones_row = cpool.tile([1, 128], f32)
            nc.gpsimd.memset(ones_row[:], 1.0)

            Wn_sb = cpool.tile([128, ND // 128, D], f32)
            nc.sync.dma_start(out=Wn_sb[:], in_=Wn_p[:].rearrange("(k p) d -> p k d", p=128))
            Wn_bf = cpool.tile([128, ND // 128, D], bft)
            nc.vector.tensor_copy(Wn_bf[:], Wn_sb[:])
            We_sb = cpool.tile([128, D], f32)
            nc.sync.dma_start(out=We_sb[:], in_=We_p[:])
            Wc1_sb = cpool.tile([128, D], f32)
            nc.sync.dma_start(out=Wc1_sb[:], in_=Wc1_p[:])
            Wc2_sb = cpool.tile([128, D], f32)
            nc.sync.dma_start(out=Wc2_sb[:], in_=Wc2_p[:])
            bias0_sb = cpool.tile([1, D], f32)
            nc.sync.dma_start(out=bias0_sb[:], in_=bias0_p[:])
            rowbnbe_sb = cpool.tile([128, D], f32)
            nc.sync.dma_start(out=rowbnbe_sb[:], in_=rowbnbe_p[:])
            rcol_sb = cpool.tile([128, NB], f32)
            nc.sync.dma_start(out=rcol_sb[:], in_=rcol_p[:])
            mcol_sb = cpool.tile([128, NB], f32)
            nc.sync.dma_start(out=mcol_sb[:], in_=mcol_p[:])

            gidx_sb = cpool.tile([128, L // 16], i16)
            nc.sync.dma_start(out=gidx_sb[:], in_=gidx_p[:])
            dstl_sb = cpool.tile([128, T_tot], i16)
            nc.sync.dma_start(out=dstl_sb[:], in_=dstl_p[:])
            dstf = cpool.tile([128, T_tot], bft)
            nc.vector.tensor_copy(dstf[:], dstl_sb[:])

            hownT_sb = tpool.tile([128, NB, BLK], f32)  # ht.T of own nodes

            # ---- Phase A: ht (bf16) -> htab, chunks of 512 nodes ----
            with (
                tc.tile_pool(name="phA", bufs=3) as apool,
                tc.tile_pool(name="psA", bufs=2, space="PSUM") as apsum,
            ):
                CH = 512
                nchunks = (N + CH - 1) // CH
                last_htab_w = None
                mid_htab_w = None
                for ci in range(nchunks):
                    n0 = ci * CH
                    P = min(CH, N - n0)
                    nf_t = apool.tile([128, 2, CH], bft, tag="nf")
                    nc.sync.dma_start(
                        out=nf_t[:, :, :P],
                        in_=nfT_b[:, n0 : n0 + P].rearrange("(k p) n -> p k n", p=128),
                    )
                    hbf = apool.tile([128, CH // 128, D], bft, tag="hbf")
                    nsub = (P + 127) // 128
                    for s in range(nsub):
                        sp = min(128, P - s * 128)
                        ps = apsum.tile([128, D], f32, tag="psA")
                        for k in range(2):
                            nc.tensor.matmul(
                                ps[:sp, :],
                                lhsT=nf_t[:, k, s * 128 : s * 128 + sp],
                                rhs=Wn_bf[:, k, :],
                                start=(k == 0),
                                stop=(k == 1),
                            )
                        nc.vector.tensor_copy(hbf[:sp, s, :], ps[:sp, :])
                    if n0 + P <= HALF:
                        ht_out = htab_lo[n0 : n0 + P, :]
                    else:
                        assert n0 >= HALF
                        ht_out = htab_hi[n0 - HALF : n0 - HALF + P, :]
                    last_htab_w = nc.sync.dma_start(
                        out=ht_out.rearrange("(t p) d -> p t d", p=128)
                        if P % 128 == 0
                        else ht_out.rearrange("(t p) d -> p t d", p=P),
                        in_=hbf[:, :nsub, :] if P % 128 == 0 else hbf[:P, :1, :],
                    )
                    if ci == 55:
                        mid_htab_w = last_htab_w

                # ---- Phase A2: ht.T of own nodes (f32, transposed layout) ----
                CH2 = 500  # 4 blocks per chunk
                for ci in range(NPC // CH2):
                    n0 = ci * CH2
                    nfo_t = apool.tile([128, 2, CH2], f32, tag="nfo")
                    d = nc.sync.dma_start(
                        out=nfo_t[:],
                        in_=nfT_own[:, n0 : n0 + CH2].rearrange("(k p) n -> p k n", p=128),
                    )
                    add_dep_helper(d.ins, last_htab_w.ins, reason="defer A2 dma past htab")
                    ps2 = apsum.tile([128, CH2], f32, tag="psA2")
                    for k in range(2):
                        nc.tensor.matmul(
                            ps2[:],
                            lhsT=Wn_sb[:, k, :],
                            rhs=nfo_t[:, k, :],
                            start=(k == 0),
                            stop=(k == 1),
                        )
                    for j in range(CH2 // BLK):
                        nc.vector.tensor_copy(
                            hownT_sb[:, ci * (CH2 // BLK) + j, :],
                            ps2[:, j * BLK : (j + 1) * BLK],
                        )

            # ---- Phase B: edge stream + per-block combine epilogue ----
            with (
                tc.tile_pool(name="phB", bufs=3) as bpool,
                tc.tile_pool(name="phC", bufs=2) as cpl,
                tc.tile_pool(name="psB", bufs=2, space="PSUM") as bpsum,
            ):
                maxTb = max(tl + th for tl, th in T_list)
                for z in range(3):
                    gz = bpool.tile([128, maxTb, D], bft, tag="gath", name=f"gz{z}")
                    nc.gpsimd.memset(gz[:], 0.0)
                toff = 0
                qi = 0
                for b in range(NB):
                    Tlo, Thi = T_list[b]
                    Tb = Tlo + Thi
                    acc_h = bpsum.tile([128, D], f32, tag="acc_h")
                    acc_e = bpsum.tile([128, D], f32, tag="acc_e")
                    eft = bpool.tile([128, Tb, D], bft, tag="eft")
                    nc.sync.dma_start(out=eft[:], in_=ef_p[:, toff : toff + Tb, :])
                    gl = bpool.tile([128, Tb, D], bft, tag="gath")
                    for c0 in range(0, Tlo, GMAX):
                        ch = min(GMAX, Tlo - c0)
                        nc.gpsimd.dma_gather(
                            gl[:, c0 : c0 + ch, :],
                            htab_lo[:],
                            gidx_sb[:, (toff + c0) * 8 : (toff + c0 + ch) * 8],
                            ch * 128, ch * 128, D,
                            queue_num=qi % NQ,
                        )
                        qi += 1
                    for c0 in range(0, Thi, GMAX):
                        ch = min(GMAX, Thi - c0)
                        nc.gpsimd.dma_gather(
                            gl[:, Tlo + c0 : Tlo + c0 + ch, :],
                            htab_hi[:],
                            gidx_sb[:, (toff + Tlo + c0) * 8 : (toff + Tlo + c0 + ch) * 8],
                            ch * 128, ch * 128, D,
                            queue_num=qi % NQ,
                        )
                        qi += 1
                    for t0 in range(0, Tb, 4):
                        w = min(4, Tb - t0)
                        S4 = bpool.tile([128, 4, 128], bft, tag="S", bufs=4, name="S4")
                        nc.vector.tensor_tensor(
                            out=S4[:, :w, :],
                            in0=dstf[:, toff + t0 : toff + t0 + w, None].to_broadcast([128, w, 128]),
                            in1=iota4[:, :w, :],
                            op=mybir.AluOpType.is_equal,
                        )
                        for j in range(w):
                            t = t0 + j
                            nc.tensor.matmul(acc_h[:], lhsT=S4[:, j, :], rhs=gl[:, t, :],
                                             start=(t == 0), stop=False)
                            nc.tensor.matmul(acc_e[:], lhsT=S4[:, j, :], rhs=eft[:, t, :],
                                             start=(t == 0), stop=(t == Tb - 1))
                    toff += Tb

                    # ---- block epilogue: combine ----
                    Tef = cpl.tile([128, D], f32, tag="Tef")
                    nc.vector.tensor_copy(Tef[:BLK, :], acc_e[:BLK, :])
                    pT = bpsum.tile([128, BLK], f32, tag="pT")
                    nc.tensor.transpose(pT[:], Tef[:BLK, :], ident[:BLK, :BLK])
                    SefT = cpl.tile([128, BLK], f32, tag="SefT")
                    nc.vector.tensor_copy(SefT[:], pT[:])
                    # acc_h[0:125,:] += Sef @ We  (same open accumulation group)
                    nc.tensor.matmul(acc_h[:BLK, :], lhsT=SefT[:], rhs=We_sb[:],
                                     start=False, stop=True, skip_group_check=True)
                    # A = acc_h * rcol + rowbnbe * mcol
                    A1 = cpl.tile([BLK, D], f32, tag="A1")
                    nc.scalar.activation(A1[:], acc_h[:BLK, :],
                                         mybir.ActivationFunctionType.Copy,
                                         scale=rcol_sb[:BLK, b : b + 1])
                    A2 = cpl.tile([BLK, D], f32, tag="A2")
                    nc.scalar.activation(A2[:], rowbnbe_sb[:BLK, :],
                                         mybir.ActivationFunctionType.Copy,
                                         scale=mcol_sb[:BLK, b : b + 1])
                    A = cpl.tile([BLK, D], f32, tag="A")
                    nc.vector.tensor_add(A[:], A1[:], A2[:])
                    pT2 = bpsum.tile([128, BLK], f32, tag="pT")
                    nc.tensor.transpose(pT2[:], A[:], ident[:BLK, :BLK])
                    AT = cpl.tile([128, BLK], f32, tag="AT")
                    nc.vector.tensor_copy(AT[:], pT2[:])
                    po = bpsum.tile([BLK, D], f32, tag="po")
                    nc.tensor.matmul(po[:], lhsT=hownT_sb[:, b, :], rhs=Wc1_sb[:],
                                     start=True, stop=False)
                    nc.tensor.matmul(po[:], lhsT=AT[:], rhs=Wc2_sb[:],
                                     start=False, stop=False)
                    nc.tensor.matmul(po[:], lhsT=ones_row[:, :BLK], rhs=bias0_sb[:],
                                     start=False, stop=True)
                    oc = cpl.tile([BLK, D], f32, tag="oc")
                    nc.vector.tensor_copy(oc[:], po[:])
                    nc.sync.dma_start(out=out_p[b * BLK : (b + 1) * BLK, :], in_=oc[:])

    nc.finalize()
    return nc


def kernel(node_feat, edge_feat, Wn, bn, We, be, Wc, bc, src, dst):
    global LAST_EXEC_NS, LAST_RESULTS
    node_feat = np.asarray(node_feat, np.float32)
    edge_feat = np.asarray(edge_feat, np.float32)
    Wn = np.asarray(Wn, np.float32)
    bn = np.asarray(bn, np.float32)
    We = np.asarray(We, np.float32)
    be = np.asarray(be, np.float32)
    Wc = np.asarray(Wc, np.float32)
    bc = np.asarray(bc, np.float32)
    src = np.asarray(src).astype(np.int64)
    dst = np.asarray(dst).astype(np.int64)

    # ---- host-side edge sharding / ordering ----
    cid = dst // NPC
    rel = dst - cid * NPC
    blk = rel // BLK
    dl = (rel - blk * BLK).astype(np.int16)
    half = (src >= HALF).astype(np.int64)
    group = (cid * NB + blk) * 2 + half          # [E] in [0, 640)
    order = np.argsort(group, kind="stable")
    counts = np.bincount(group, minlength=C * NB * 2).reshape(C, NB, 2)
    tcnt = (counts + 127) // 128
    Tmax = tcnt.max(axis=0)                       # [NB, 2]
    T_list = [(int(Tmax[b, 0]), int(Tmax[b, 1])) for b in range(NB)]
    T_tot = int(Tmax.sum())
    L = T_tot * 128

    deg = np.bincount(dst, minlength=N).astype(np.float32)
    rcol_all = 1.0 / np.maximum(deg, 1.0)
    mcol_all = np.minimum(deg, 1.0)

    ef_bf = edge_feat.astype(bf16)

    gstart = np.zeros(C * NB * 2 + 1, np.int64)
    np.cumsum(counts.ravel(), out=gstart[1:])
    slot_off = np.zeros(NB * 2 + 1, np.int64)
    np.cumsum((Tmax.ravel() * 128), out=slot_off[1:])

    in_maps = []
    shared = {
        "nfT_b": np.ascontiguousarray(node_feat.T.astype(bf16)),
        "Wn": Wn,
        "We": We,
        "Wc1": np.ascontiguousarray(Wc[:D]),
        "Wc2": np.ascontiguousarray(Wc[D:]),
        "bias0": (bn @ Wc[:D] + bc).reshape(1, D),
        "rowbnbe": np.tile((bn + be).reshape(1, D), (128, 1)),
    }
    for c in range(C):
        gidx = np.zeros(L, np.int16)
        dstl = np.full(L, PAD_COL, np.int16)
        eids = np.full(L, -1, np.int64)
        for b in range(NB):
            for h in range(2):
                g = (c * NB + b) * 2 + h
                n = counts[c, b, h]
                s0 = gstart[g]
                o0 = slot_off[b * 2 + h]
                ed = order[s0 : s0 + n]
                gidx[o0 : o0 + n] = (src[ed] - h * HALF).astype(np.int16)
                dstl[o0 : o0 + n] = dl[ed]
                eids[o0 : o0 + n] = ed
        ef_rows = np.zeros((L, D), bf16)
        real = eids >= 0
        ef_rows[real] = ef_bf[eids[real]]
        rc = np.zeros((128, NB), np.float32)
        mc = np.zeros((128, NB), np.float32)
        for b in range(NB):
            n0 = c * NPC + b * BLK
            rc[:BLK, b] = rcol_all[n0 : n0 + BLK]
            mc[:BLK, b] = mcol_all[n0 : n0 + BLK]
        in_maps.append(
            dict(
                shared,
                nfT_own=np.ascontiguousarray(node_feat.T[:, c * NPC : (c + 1) * NPC]),
                rcol=rc,
                mcol=mc,
                gidx=_wrap_idx16(gidx),
                dstl=np.ascontiguousarray(dstl.reshape(T_tot, 128).T),
                ef=np.ascontiguousarray(
                    ef_rows.reshape(T_tot, 128, D).transpose(1, 0, 2)
                ),
            )
        )

    nc = _build_graph(T_list)
    res = run_bass_kernel_spmd(nc, in_maps, core_ids=list(range(C)))
    LAST_EXEC_NS = res.exec_time_ns
    LAST_RESULTS = res
    out = np.concatenate([res.results[c]["out"] for c in range(C)], axis=0)
    return out

